# revision 23
# baseline (speedup 1.0000x reference)
"""Distributed causal multi-head attention for Trainium2 (8 NeuronCores).

Problem (hardcoded): x[2, 2048, 1024], 16 heads, head_dim 64, causal
softmax(QK^T/8)V then out-proj with bias. f32 in/out.

Sharding: data parallel on batch (cores 0-3 -> batch 0, 4-7 -> batch 1),
tensor parallel on heads within each group of 4 (4 heads per core).
Each core:
  - computes Q^T,K^T (head pairs packed to 128 partitions), V for its 4 heads
  - scores transposed S^T[k,q] = K Q^T so the softmax denominator comes out
    of the PE via an appended ones-column on V (no partition reductions)
  - exp without max-subtraction (scores are O(2), safe in fp32/bf16)
  - causal mask applied post-exp as a 0/1 bf16 multiply (DVE 4x mode)
  - ctx^T accumulated per q-chunk, normalized with 1/den partition-broadcast
  - AllGather of ctx^T bf16 [256,2048] within the 4-core group
  - column-parallel out-proj: outT[oc,q] = Wo[:,oc]^T ctxT + bo[oc]
Host assembles out[b, :, oc_slice] from each core's outT.

All matmuls bf16 (fp32 PSUM accumulation): measured end-to-end rel err
(Frobenius) ~3e-3 vs the f32 reference.
"""

import numpy as np
import ml_dtypes

from concourse import bass, bacc, mybir
from concourse import tile
from concourse.bass_utils import run_bass_kernel_spmd

BF16 = mybir.dt.bfloat16
F32 = mybir.dt.float32
Act = mybir.ActivationFunctionType

B, S, D = 2, 2048, 1024
H, HD = 16, 64
NCORES = 8
GROUP = 4            # cores per batch group
HPC = H // GROUP     # 4 heads per core
CW = HPC * HD        # 256 columns per core
QC = 512             # q-chunk width
KC = 128             # k-chunk width
NQ = S // QC         # 4
NKC = S // KC        # 16
KPQ = QC // KC       # 4 k-chunks per q-chunk
DCH = D // 128       # 8 contraction chunks of 128

_CACHE = {}


def _build_bass():
    nc = bacc.Bacc(
        "TRN2", target_bir_lowering=False, debug=False, num_devices=NCORES
    )

    # per-core external inputs (same shapes on every core: SPMD)
    xT = nc.declare_dram_parameter("xT", [D, S], BF16, isOutput=False)
    wq = nc.declare_dram_parameter("wq", [D, CW], BF16, isOutput=False)
    wk = nc.declare_dram_parameter("wk", [D, CW], BF16, isOutput=False)
    wv = nc.declare_dram_parameter("wv", [D, CW], BF16, isOutput=False)
    wo = nc.declare_dram_parameter("wo", [D, CW], BF16, isOutput=False)
    bo = nc.declare_dram_parameter("bo", [CW, 1], F32, isOutput=False)
    msk = nc.declare_dram_parameter("msk", [128, KPQ, QC], BF16, isOutput=False)
    vones = nc.declare_dram_parameter("vones", [128, NKC, HPC, 1], BF16, isOutput=False)
    # selector for den broadcast: bc[m,q] = sum_k sel33[k,m]*den_pair[k,q]
    sel33 = nc.declare_dram_parameter("sel33", [33, 128], BF16, isOutput=False)
    outT = nc.declare_dram_parameter("outT", [CW, S], F32, isOutput=True)

    with tile.TileContext(nc) as tc:
        with tc.tile_pool(name="dram", bufs=1, space="DRAM") as dram:
            cc_in = dram.tile([CW, S], BF16, name="cc_in")
            # Shared addr_space needs >4-core groups; Local is required here.
            cc_out = dram.tile([D, S], BF16, name="cc_out")

            with tc.tile_pool(name="persist", bufs=1) as pp:
                # lives across the whole kernel: ~92 KB/partition
                wq_sb = pp.tile([128, DCH, CW], BF16, tag="wq_sb")
                wk_sb = pp.tile([128, DCH, CW], BF16, tag="wk_sb")
                wv_sb = pp.tile([128, DCH, CW], BF16, tag="wv_sb")
                wo_sb = pp.tile([128, DCH, CW], BF16, tag="wo_sb")
                bo_sb = pp.tile([128, CW // 128, 1], F32, tag="bo_sb")
                msk_sb = pp.tile([128, KPQ, QC], BF16, tag="msk_sb")
                qT_sb = pp.tile([128, 2, S], BF16, tag="qT_sb")
                kT_sb = pp.tile([128, 2, S], BF16, tag="kT_sb")
                v_aug = pp.tile([128, NKC, HPC, HD + 1], BF16, tag="v_aug")
                ctxu0 = pp.tile([128, S], F32, tag="ctxu0")
                ctxu1 = pp.tile([128, S], F32, tag="ctxu1")
                # den per pair: head 2p at partition 0, head 2p+1 at partition
                # 32 (ACT writes must start at multiples of 32); rows 1-31 are
                # zeroed so the K=33 selector matmul can broadcast both heads
                # to output partitions 0-63 / 64-127 in one instruction
                den_pair = [pp.tile([33, S], BF16, tag=f"den{p}", name=f"den{p}")
                            for p in range(2)]
                sel_sb = pp.tile([33, 128], BF16, tag="sel_sb")
                ctxu_pair = [ctxu0, ctxu1]
                for p in range(2):
                    nc.vector.memset(den_pair[p][:], 0.0)

                for w_sb, w in ((wq_sb, wq), (wk_sb, wk), (wv_sb, wv), (wo_sb, wo)):
                    for c in range(DCH):
                        nc.sync.dma_start(w_sb[:, c, :], w[c * 128:(c + 1) * 128, :])
                for o in range(CW // 128):
                    nc.sync.dma_start(bo_sb[:, o, :], bo[o * 128:(o + 1) * 128, :])
                nc.sync.dma_start(msk_sb[:], msk[:])
                # ones column of V_aug comes from the host: keeps the V
                # PSUM->SBUF copy to a single (PE) sync wait
                nc.sync.dma_start(v_aug[:, :, :, HD:HD + 1], vones[:])
                nc.sync.dma_start(sel_sb[:], sel33[:])

                # ---- phase 1: projections (xT resident only here) ----
                with tc.tile_pool(name="xpool", bufs=1) as xp, \
                     tc.tile_pool(name="proj_ps", bufs=3, space="PSUM") as projp:
                    xT_sb = xp.tile([128, DCH, S], BF16, tag="xT_sb")
                    for c in range(DCH):
                        nc.sync.dma_start(xT_sb[:, c, :], xT[c * 128:(c + 1) * 128, :])

                    # Q^T / K^T: out [128 (2 heads), S] per pair
                    for w_sb, dst in ((wq_sb, qT_sb), (wk_sb, kT_sb)):
                        for pair in range(2):
                            for j in range(NQ):
                                ps = projp.tile([128, QC], F32, tag="proj")
                                for c in range(DCH):
                                    nc.tensor.matmul(
                                        ps[:],
                                        w_sb[:, c, pair * 128:(pair + 1) * 128],
                                        xT_sb[:, c, j * QC:(j + 1) * QC],
                                        start=(c == 0),
                                        stop=(c == DCH - 1),
                                    )
                                nc.scalar.activation(
                                    dst[:, pair, j * QC:(j + 1) * QC], ps[:],
                                    Act.Identity,
                                )
                    # V: [tok, 4 heads * (64+1)] with ones column
                    for t in range(NKC):
                        ps = projp.tile([128, CW], F32, tag="projv")
                        for c in range(DCH):
                            nc.tensor.matmul(
                                ps[:],
                                xT_sb[:, c, t * 128:(t + 1) * 128],
                                wv_sb[:, c, :],
                                start=(c == 0),
                                stop=(c == DCH - 1),
                            )
                        for h in range(HPC):
                            nc.scalar.activation(
                                v_aug[:, t, h, 0:HD],
                                ps[:, h * HD:(h + 1) * HD],
                                Act.Identity,
                            )

                # ---- phase 2: attention, head by head ----
                with tc.tile_pool(name="sc_ps", bufs=3, space="PSUM") as scp, \
                     tc.tile_pool(name="ct_ps", bufs=2, space="PSUM") as ctp, \
                     tc.tile_pool(name="es_pool", bufs=NKC + 2) as esp:
                    for h in range(HPC):
                        pair, row = h // 2, (h % 2) * 64
                        for j in range(NQ):
                            nkc = (j + 1) * KPQ
                            qs = slice(j * QC, (j + 1) * QC)
                            es_tiles = []
                            for c in range(nkc):
                                st = scp.tile([128, QC], F32, tag="st")
                                nc.tensor.matmul(
                                    st[:],
                                    kT_sb[row:row + 64, pair, c * KC:(c + 1) * KC],
                                    qT_sb[row:row + 64, pair, qs],
                                    start=True, stop=True,
                                )
                                es = esp.tile([128, QC], BF16, tag="es")
                                nc.scalar.activation(es[:], st[:], Act.Exp, scale=0.125)
                                if c >= j * KPQ:
                                    r = c - j * KPQ
                                    nc.vector.tensor_mul(es[:], es[:], msk_sb[:, r, :])
                                es_tiles.append(es)
                            ct = ctp.tile([HD + 1, QC], F32, tag="ct")
                            for c in range(nkc):
                                nc.tensor.matmul(
                                    ct[:],
                                    v_aug[:, c, h, :],
                                    es_tiles[c][:],
                                    start=(c == 0),
                                    stop=(c == nkc - 1),
                                )
                            nc.scalar.activation(
                                ctxu_pair[pair][row:row + 64, qs], ct[0:HD, :],
                                Act.Identity,
                            )
                            nc.scalar.activation(
                                den_pair[pair][(h % 2) * 32:(h % 2) * 32 + 1, qs],
                                ct[HD:HD + 1, :], Act.Identity,
                            )

                # ---- phase 3: normalize + feed collective ----
                with tc.tile_pool(name="bc_ps", bufs=3, space="PSUM") as bcp, \
                     tc.tile_pool(name="norm", bufs=2) as np_pool:
                    for pair in range(2):
                        ctxn = np_pool.tile([128, S], BF16, tag="ctxn")
                        for j in range(NQ):
                            qs = slice(j * QC, (j + 1) * QC)
                            bc = bcp.tile([128, QC], F32, tag="bc")
                            nc.tensor.matmul(
                                bc[:], sel_sb[:], den_pair[pair][:, qs],
                                start=True, stop=True,
                            )
                            rb = np_pool.tile([128, QC], F32, tag="rb")
                            nc.vector.reciprocal(rb[:], bc[:])
                            nc.vector.tensor_mul(
                                ctxn[:, qs], ctxu_pair[pair][:, qs], rb[:]
                            )
                        nc.sync.dma_start(
                            cc_in[pair * 128:(pair + 1) * 128, :], ctxn[:]
                        )

                nc.gpsimd.collective_compute(
                    "AllGather",
                    mybir.AluOpType.bypass,
                    replica_groups=[[0, 1, 2, 3], [4, 5, 6, 7]],
                    ins=[cc_in.opt()],
                    outs=[cc_out.opt()],
                )

                # ---- phase 4: out-proj outT[oc, q] = Wo[:, oc]^T ctxT + bo ----
                with tc.tile_pool(name="cpool", bufs=1) as cp, \
                     tc.tile_pool(name="out_ps", bufs=3, space="PSUM") as outp, \
                     tc.tile_pool(name="out_sb", bufs=3) as outs:
                    ctxT_sb = cp.tile([128, DCH, S], BF16, tag="ctxT_sb")
                    for c in range(DCH):
                        nc.sync.dma_start(
                            ctxT_sb[:, c, :], cc_out[c * 128:(c + 1) * 128, :]
                        )
                    for o in range(CW // 128):
                        for j in range(NQ):
                            ps = outp.tile([128, QC], F32, tag="ops")
                            for c in range(DCH):
                                nc.tensor.matmul(
                                    ps[:],
                                    wo_sb[:, c, o * 128:(o + 1) * 128],
                                    ctxT_sb[:, c, j * QC:(j + 1) * QC],
                                    start=(c == 0),
                                    stop=(c == DCH - 1),
                                )
                            ot = outs.tile([128, QC], F32, tag="ot")
                            nc.scalar.activation(
                                ot[:], ps[:], Act.Identity, bias=bo_sb[:, o, :]
                            )
                            nc.sync.dma_start(
                                outT[o * 128:(o + 1) * 128, j * QC:(j + 1) * QC],
                                ot[:],
                            )
    nc.compile()
    return nc


def _causal_mask():
    # msk[kp, r, qf] = 1 where (r*128 + kp) <= qf else 0  (keep k <= q)
    kp = np.arange(128)[:, None, None]
    r = np.arange(KPQ)[None, :, None]
    qf = np.arange(QC)[None, None, :]
    return (r * 128 + kp <= qf).astype(ml_dtypes.bfloat16)


def _in_maps(x, Wq, Wk, Wv, Wo, bo):
    bf = ml_dtypes.bfloat16
    msk = _causal_mask()
    sel33 = np.zeros((33, 128), dtype=bf)
    sel33[0, 0:64] = 1.0
    sel33[32, 64:128] = 1.0
    xT = [np.ascontiguousarray(x[b].T).astype(bf) for b in range(B)]
    maps = []
    for c in range(NCORES):
        b, g = c // GROUP, c % GROUP
        cs = slice(g * CW, (g + 1) * CW)
        maps.append({
            "xT": xT[b],
            "wq": np.ascontiguousarray(Wq[:, cs]).astype(bf),
            "wk": np.ascontiguousarray(Wk[:, cs]).astype(bf),
            "wv": np.ascontiguousarray(Wv[:, cs]).astype(bf),
            "wo": np.ascontiguousarray(Wo[:, cs]).astype(bf),
            "bo": np.ascontiguousarray(bo[cs, None]).astype(np.float32),
            "msk": msk,
            "vones": np.ones((128, NKC, HPC, 1), dtype=bf),
            "sel33": sel33,
        })
    return maps


def kernel(x, Wq, Wk, Wv, Wo, bo, _trace=False):
    x = np.asarray(x, dtype=np.float32)
    Wq, Wk, Wv, Wo, bo = (np.asarray(a, dtype=np.float32) for a in (Wq, Wk, Wv, Wo, bo))
    if "nc" not in _CACHE:
        _CACHE["nc"] = _build_bass()
    nc = _CACHE["nc"]
    res = run_bass_kernel_spmd(
        nc, _in_maps(x, Wq, Wk, Wv, Wo, bo), list(range(NCORES)), trace=_trace
    )
    out = np.zeros((B, S, D), dtype=np.float32)
    for c in range(NCORES):
        b, g = c // GROUP, c % GROUP
        out[b, :, g * CW:(g + 1) * CW] = res.results[c]["outT"].T
    if _trace:
        return out, res
    return out
